# revision 25
# baseline (speedup 1.0000x reference)
"""Trainium2 Bass kernel for nn_Block_3951369912372 (dense transformer block).

Reference computation (per batch element b of 8, handled by core b):
  x: [4, 512, 768]  (S=4 groups of N=512 tokens, D=768)
  h   = LN(x; g1, b1)
  qkv = h @ Wqkv                      (12 heads, head_dim 64)
  attn over the N=512 tokens within each s-group, per head
  y   = attn_out @ Wp + bp;  x2 = x + y
  h2  = LN(x2; g2, b2)
  out = x2 + gelu(h2 @ W1 + bm1) @ W2 + bm2

Strategy: data-parallel over B across the 8 cores; within a core all
activations live d-major ([feature, token]) so every matmul contracts the
feature dim on partitions.  All matmuls run in bf16 with fp32 PSUM
accumulation.  LN stats via ones-column matmuls; row broadcasts via a DMA
round-trip through DRAM (this walrus can't encode the gpsimd
partition_broadcast custom op).  Softmax skips max-subtraction (|scores| < 8
for the target distribution) and gets denominators for free from a ones
column appended to V.  The s-blocks are software-pipelined: attention of
block sb+1 runs between proj(sb) and MLP(sb) so LayerNorm chains never
stall the PE.
"""

import numpy as np
import ml_dtypes

import bass_rust
import concourse.bass as bass
import concourse.mybir as mybir
import concourse.tile as tile
from concourse.bass_utils import run_bass_kernel_spmd
from concourse.vector_clock import ScopedClock

F32 = mybir.dt.float32
BF16 = mybir.dt.bfloat16
AF = mybir.ActivationFunctionType
OP = mybir.AluOpType

DIM = 768
HEADS = 12
HD = 64
HIDDEN = 3072
EPS = 1e-5
SCALE = HD ** -0.5
S = 4
N = 512
TOK = S * N          # tokens per core
P = 128
NCH = DIM // P       # 6 feature chunks
KCH = HIDDEN // P    # 24 hidden chunks
NPAIR = HEADS // 2   # 6 head pairs


class FixedTileContext(tile.TileContext):
    """Walrus in this container rejects instructions with more than ~1 sem
    wait ("Too many sync wait commands").  After scheduling, spread excess
    waits onto same-engine carrier nops inserted immediately before the
    over-limit instruction."""

    CAP = 1

    def _drain_and_barrier(self, tick_clock, wait_clock):
        super()._drain_and_barrier(tick_clock, wait_clock)
        nc = self.nc
        # Drop Ldweights that reload the exact weights already resident in
        # the PE array (same AP/config as the immediately preceding
        # Ldweights, with no sync side effects).
        for bb in nc.m.functions[0].blocks:
            prev_key = None
            newlist = []
            for inst in bb.instructions:
                tn = type(inst).__name__
                if tn == "InstLdweights":
                    si = inst.sync_info
                    clean = si is None or (not si.on_wait and not si.on_update)
                    key = (str(inst.ins),
                           str(getattr(inst, "perf_mode", None)),
                           str(getattr(inst, "is_transpose", None)),
                           str(getattr(inst, "tile_position", None)))
                    if clean and key == prev_key:
                        continue  # redundant reload
                    if clean:
                        prev_key = key
                    else:
                        prev_key = None
                elif tn not in ("InstMatmult",):
                    pass  # non-PE instructions don't disturb PE weights
                newlist.append(inst)
            bb.instructions = newlist
        nfix = 0
        for bb in nc.m.functions[0].blocks:
            newlist = []
            changed = False
            for inst in bb.instructions:
                si = inst.sync_info
                waits = list(si.on_wait) if si is not None else []
                if len(waits) > self.CAP:
                    for w in waits[:-self.CAP]:
                        nop = mybir.InstNoOp(
                            name=f"I-waitfix-{nfix}",
                            sync_info=bass_rust.SyncInfo(
                                on_wait=[w], on_update=[]),
                            bass_nofuse=True,
                            engine=inst.engine,
                        )
                        nfix += 1
                        nc.register_instruction(nop, overwrite=True)
                        newlist.append(nop)
                    si.on_wait = waits[-self.CAP:]
                    changed = True
                newlist.append(inst)
            if changed:
                bb.instructions = newlist


def build_bass(debug=False, reps=1):
    nc = bass.Bass("TRN2", target_bir_lowering=False, debug=False, num_devices=8)

    xT = nc.dram_tensor("xT", [DIM, TOK], F32, kind="ExternalInput")
    # weights, host-prepared in SBUF layout [p, chunk, cols]
    wq = nc.dram_tensor("wq", [P, NCH, DIM], BF16, kind="ExternalInput")
    wk = nc.dram_tensor("wk", [P, NCH, DIM], BF16, kind="ExternalInput")
    wv = nc.dram_tensor("wv", [P, NCH, DIM], BF16, kind="ExternalInput")
    wp = nc.dram_tensor("wp", [P, NCH, DIM], BF16, kind="ExternalInput")
    # streamed weights, pre-tiled so each tile is contiguous in DRAM
    w1 = nc.dram_tensor("w1", [KCH, P, NCH, P], BF16, kind="ExternalInput")
    w2 = nc.dram_tensor("w2", [NCH, P, KCH, P], BF16, kind="ExternalInput")
    # per-feature biases, [p, chunk] layout
    bq = nc.dram_tensor("bq", [P, NCH], F32, kind="ExternalInput")
    bk = nc.dram_tensor("bk", [P, NCH], F32, kind="ExternalInput")
    bv = nc.dram_tensor("bv", [P, NCH], F32, kind="ExternalInput")
    bp = nc.dram_tensor("bp", [P, NCH], F32, kind="ExternalInput")
    bm1 = nc.dram_tensor("bm1", [P, KCH], F32, kind="ExternalInput")
    bm2 = nc.dram_tensor("bm2", [P, NCH], F32, kind="ExternalInput")

    out = nc.dram_tensor("out", [DIM, TOK], F32, kind="ExternalOutput")
    dbg = {}
    if debug:
        for nm, shp in [
            ("d_hT", [DIM, TOK]), ("d_V", [TOK, HEADS * 65]),
            ("d_qk", [2 * DIM, TOK]), ("d_PT", [N, N]),
            ("d_yT", [DIM, TOK]), ("d_x2T", [DIM, TOK]), ("d_mT", [HIDDEN, TOK]),
        ]:
            dbg[nm] = nc.dram_tensor(nm, shp, F32, kind="ExternalOutput")

    xT_a = xT.ap().rearrange("(c p) t -> p c t", p=P)
    out_a = out.ap().rearrange("(c p) t -> p c t", p=P)

    with FixedTileContext(nc) as tc:
        args = (nc, tc, xT_a, out_a,
                wq.ap(), wk.ap(), wv.ap(), wp.ap(), w1.ap(), w2.ap(),
                bq.ap(), bk.ap(), bv.ap(), bp.ap(), bm1.ap(), bm2.ap(),
                {k: v.ap() for k, v in dbg.items()})
        for _ in range(reps):
            _body(*args)
    return nc


PHASE_LOG = []


def _body(nc, tc, xT_a, out_a, wq_a, wk_a, wv_a, wp_a, w1_a, w2_a,
          bq_a, bk_a, bv_a, bp_a, bm1_a, bm2_a, dbg):
    ctx_pools = {}

    def pool(name, bufs, space="SBUF"):
        p = tc.alloc_tile_pool(name=name, bufs=bufs, space=space)
        ctx_pools[name] = p
        return p

    # ---- persistent (bufs=1) ----
    singles = pool("singles", 1)
    wq_sb = singles.tile([P, NCH, DIM], BF16, tag="wq")
    wk_sb = singles.tile([P, NCH, DIM], BF16, tag="wk")
    wv_sb = singles.tile([P, NCH, DIM], BF16, tag="wv")
    wp_sb = singles.tile([P, NCH, DIM], BF16, tag="wp")
    bq_sb = singles.tile([P, NCH], F32, tag="bq")
    bk_sb = singles.tile([P, NCH], F32, tag="bk")
    bv_sb = singles.tile([P, NCH], F32, tag="bv")
    bp_sb = singles.tile([P, NCH], F32, tag="bp")
    bm1_sb = singles.tile([P, KCH], F32, tag="bm1")
    bm2_sb = singles.tile([P, NCH], F32, tag="bm2")
    nc.sync.dma_start(bq_sb[:], bq_a)
    nc.sync.dma_start(bk_sb[:], bk_a)
    nc.sync.dma_start(bv_sb[:], bv_a)
    nc.sync.dma_start(bp_sb[:], bp_a)
    nc.sync.dma_start(bm1_sb[:], bm1_a)
    nc.sync.dma_start(bm2_sb[:], bm2_a)
    ones_col = singles.tile([P, 1], BF16, tag="ones")
    nc.vector.memset(ones_col[:], 1.0)
    eps_tile = singles.tile([1, 1], F32, tag="eps")
    nc.vector.memset(eps_tile[:], EPS)

    # ---- pools ----
    xT_p = pool("xT", 2)          # [P, NCH, N] f32
    xb_p = pool("xb", 4)          # [P, N] bf16 per-chunk stats staging
    rows_p = pool("rows", 4)      # [1, N] stat rows
    bcast_p = pool("bcast", 2)    # [P, N] f32 broadcast rows
    hT_p = pool("hT", 2)          # [P, NCH, N] bf16
    v_p = pool("V", 4)            # [P, HEADS, 65] bf16 per tok-tile
    qk_p = pool("qk", 3)          # [P, N] bf16 (q-pair / k-pair)
    pt_p = pool("PT", 2)          # [P, 4, N] bf16 exp(scores^T)
    rb_p = pool("rb", 2)          # [HD, N] f32 recip-denominator broadcast
    yT_p = pool("yT", 2)          # [P, NCH, N] bf16
    x2T_p = pool("x2T", 2)        # [P, NCH, N] f32
    h2T_p = pool("h2T", 1)        # [P, NCH, N] bf16
    w1_p = pool("w1t", 2)         # [P, NCH, P] streamed W1 tile
    w2_p = pool("w2t", 2)         # [P, KCH, P] streamed W2 column
    mT_p = pool("mT", 1)          # [P, KCH, N] bf16
    outp = pool("outT", 2)        # [P, N] f32
    # PSUM: scores 2x[128,1024](4) + mm 2x[128,512](2) + acc 2x[128,512](2) = 8 banks
    ps_sc = pool("ps_sc", 2, space="PSUM")
    ps_mm = pool("ps_mm", 2, space="PSUM")
    ps_acc = pool("ps_acc", 2, space="PSUM")
    dram_p = pool("drows", 6, space="DRAM")

    def broadcast_row(row_ap, parts, dst_pool, tag):
        """[1, N] SBUF/PSUM row -> [parts, N] SBUF via DRAM round-trip
        (walrus here can't encode the gpsimd partition_broadcast custom op)."""
        dr = dram_p.tile([1, N], F32, tag="dr")
        nc.sync.dma_start(dr[:], row_ap)
        bt = dst_pool.tile([parts, N], F32, tag=tag)
        nc.gpsimd.dma_start(bt[:], dr[:].to_broadcast((parts, N)))
        return bt

    def layernorm(src_tile, dst_pool):
        """src [P, NCH, N] f32 -> normalized bf16 [P, NCH, N] (no affine)."""
        st = ps_mm.tile([P, N], F32, tag="mm")
        st2 = ps_mm.tile([P, N], F32, tag="mm")
        for c in range(NCH):
            xc = xb_p.tile([P, N], BF16, tag="xb")
            nc.scalar.activation(xc[:], src_tile[:, c, :], AF.Copy)
            nc.tensor.matmul(st[0:1, :], ones_col[:], xc[:],
                             start=(c == 0), stop=(c == NCH - 1))
            sc_ = xb_p.tile([P, N], BF16, tag="xb")
            nc.scalar.activation(sc_[:], src_tile[:, c, :], AF.Square)
            nc.tensor.matmul(st2[0:1, :], ones_col[:], sc_[:],
                             start=(c == 0), stop=(c == NCH - 1))
        mu = rows_p.tile([1, N], F32, tag="row")
        nc.vector.tensor_scalar_mul(mu[:], st[0:1, :], 1.0 / DIM)
        ex2 = rows_p.tile([1, N], F32, tag="row")
        nc.vector.tensor_scalar_mul(ex2[:], st2[0:1, :], 1.0 / DIM)
        var = rows_p.tile([1, N], F32, tag="row")
        nc.vector.tensor_tensor(var[:], mu[:], mu[:], OP.mult)
        nc.vector.tensor_sub(var[:], ex2[:], var[:])
        std = rows_p.tile([1, N], F32, tag="row")
        nc.scalar.activation(std[:], var[:], AF.Sqrt, bias=eps_tile[:])
        rstd = rows_p.tile([1, N], F32, tag="row")
        nc.vector.reciprocal(rstd[:], std[:])
        nmr = rows_p.tile([1, N], F32, tag="row")
        nc.vector.scalar_tensor_tensor(nmr[:], mu[:], -1.0, rstd[:], OP.mult, OP.mult)
        dr = dram_p.tile([1, 2 * N], F32, tag="dr2")
        nc.sync.dma_start(dr[:, 0:N], rstd[:])
        nc.sync.dma_start(dr[:, N:2 * N], nmr[:])
        bt = bcast_p.tile([P, 2, N], F32, tag="bc")
        nc.gpsimd.dma_start(bt[:], dr[:].rearrange("o (x t) -> o x t", t=N)
                            .to_broadcast((P, 2, N)))
        rstd_b = bt[:, 0, :]
        nmr_b = bt[:, 1, :]
        dst = dst_pool.tile([P, NCH, N], BF16, tag="h")
        nc.vector.tensor_tensor(
            dst[:], src_tile[:], rstd_b[:, None, :].to_broadcast((P, NCH, N)),
            OP.mult)
        nc.vector.tensor_tensor(
            dst[:], dst[:], nmr_b[:, None, :].to_broadcast((P, NCH, N)), OP.add)
        return dst

    def ln1_phase_inner(sb, *a):
        return _ln1_phase_impl(sb, *a)

    def ln1_phase(sb, *a, **kw):
        start = nc.next_id_peek() if hasattr(nc, 'next_id_peek') else None
        PHASE_LOG.append(("ln1_phase", sb, nc.get_next_instruction_name()))
        r = _ln1_phase_impl(sb, *a, **kw)
        return r

    def _ln1_phase_impl(sb):
        sl = slice(sb * N, (sb + 1) * N)
        xT_s = xT_p.tile([P, NCH, N], F32, tag="x")
        nc.sync.dma_start(xT_s[:], xT_a[:, :, sl])
        hT = layernorm(xT_s, hT_p)
        if dbg:
            nc.gpsimd.dma_start(
                dbg["d_hT"].rearrange("(c p) t -> p c t", p=P)[:, :, sl], hT[:])
        return xT_s, hT

    def attn_phase_inner(sb, *a):
        return _attn_phase_impl(sb, *a)

    def attn_phase(sb, *a, **kw):
        start = nc.next_id_peek() if hasattr(nc, 'next_id_peek') else None
        PHASE_LOG.append(("attn_phase", sb, nc.get_next_instruction_name()))
        r = _attn_phase_impl(sb, *a, **kw)
        return r

    def _attn_phase_impl(sb, hT):
        sl = slice(sb * N, (sb + 1) * N)
        # V = h^T.T @ Wv  (token-major, + ones column per head)
        v_tiles = []
        for tt in range(S):
            v3 = v_p.tile([P, HEADS, 65], BF16, tag="v")
            nc.vector.memset(v3[:, :, 64:65], 1.0)
            pvs = [ps_mm.tile([P, N], F32, tag="mm", name=f"pv{_h}") for _h in range(2)]
            for c in range(NCH):
                for half in range(2):
                    nc.tensor.matmul(
                        pvs[half][:, 0:384],
                        hT[:, c, tt * P:(tt + 1) * P],
                        wv_sb[:, c, half * 384:(half + 1) * 384],
                        start=(c == 0), stop=(c == NCH - 1))
            for half in range(2):
                nc.scalar.activation(
                    v3[:, half * 6:(half + 1) * 6, 0:64],
                    pvs[half][:, 0:384].rearrange("p (h d) -> p h d", d=64),
                    AF.Copy)
            v_tiles.append(v3)
        if dbg:
            for tt in range(S):
                nc.gpsimd.dma_start(
                    dbg["d_V"].rearrange("(s p) hd -> s p hd", p=P)[sb * S + tt],
                    v_tiles[tt][:].rearrange("p h d -> p (h d)"))

        yT = yT_p.tile([P, NCH, N], BF16, tag="y")
        for j in range(NPAIR):
            q_ps = ps_mm.tile([P, N], F32, tag="mm")
            for c in range(NCH):
                nc.tensor.matmul(q_ps[:], wq_sb[:, c, j * P:(j + 1) * P],
                                 hT[:, c, :], start=(c == 0), stop=(c == NCH - 1))
            q_sb = qk_p.tile([P, N], BF16, tag="qk")
            nc.vector.tensor_scalar_add(q_sb[:], q_ps[:], scalar1=bq_sb[:, j:j + 1])
            k_ps = ps_mm.tile([P, N], F32, tag="mm")
            for c in range(NCH):
                nc.tensor.matmul(k_ps[:], wk_sb[:, c, j * P:(j + 1) * P],
                                 hT[:, c, :], start=(c == 0), stop=(c == NCH - 1))
            k_sb = qk_p.tile([P, N], BF16, tag="qk")
            nc.vector.tensor_scalar_add(k_sb[:], k_ps[:], scalar1=bk_sb[:, j:j + 1])
            if dbg:
                qka = dbg["d_qk"].rearrange("(x p) t -> x p t", p=P)
                nc.gpsimd.dma_start(qka[2 * j, :, sl], q_sb[:])
                nc.gpsimd.dma_start(qka[2 * j + 1, :, sl], k_sb[:])

            pts = [pt_p.tile([P, S, N], BF16, tag="pt", name=f"pt{_h}") for _h in range(2)]
            for mh in range(2):
                scs = [ps_sc.tile([P, 2 * N], F32, tag="sc", name=f"sc{_h}") for _h in range(2)]
                for mt in range(2):
                    m = mh * 2 + mt
                    # (0,0) and (64,0) row-group matmuls run concurrently
                    for hh in range(2):
                        h0 = hh * HD
                        nc.tensor.matmul(
                            scs[hh][:, mt * N:(mt + 1) * N],
                            k_sb[h0:h0 + HD, m * P:(m + 1) * P],
                            q_sb[h0:h0 + HD, :],
                            start=True, stop=True)
                for hh in range(2):
                    nc.scalar.activation(
                        pts[hh][:, mh * 2:(mh + 1) * 2, :],
                        scs[hh][:].rearrange("p (m t) -> p m t", t=N),
                        AF.Exp, scale=SCALE)
            for hh in range(2):
                head = 2 * j + hh
                pt = pts[hh]
                if dbg and sb == 0 and head == 0:
                    nc.gpsimd.dma_start(
                        dbg["d_PT"].rearrange("(m p) t -> p m t", p=P), pt[:])
                av = ps_acc.tile([P, N], F32, tag="acc")
                for mt in range(S):
                    nc.tensor.matmul(av[0:65, :], v_tiles[mt][:, head, :],
                                     pt[:, mt, :], start=(mt == 0), stop=(mt == 3))
                rcp = rows_p.tile([1, N], F32, tag="row")
                nc.vector.reciprocal(rcp[:], av[64:65, :])
                rb = broadcast_row(rcp[:], HD, rb_p, "rb")
                nc.vector.tensor_tensor(
                    yT[(head % 2) * HD:(head % 2) * HD + HD, head // 2, :],
                    av[0:HD, :], rb[:], OP.mult)
        if dbg:
            nc.gpsimd.dma_start(
                dbg["d_yT"].rearrange("(c p) t -> p c t", p=P)[:, :, sl], yT[:])
        return yT

    def proj_phase_inner(sb, *a):
        return _proj_phase_impl(sb, *a)

    def proj_phase(sb, *a, **kw):
        start = nc.next_id_peek() if hasattr(nc, 'next_id_peek') else None
        PHASE_LOG.append(("proj_phase", sb, nc.get_next_instruction_name()))
        r = _proj_phase_impl(sb, *a, **kw)
        return r

    def _proj_phase_impl(sb, yT, xT_s):
        sl = slice(sb * N, (sb + 1) * N)
        x2T = x2T_p.tile([P, NCH, N], F32, tag="x2")
        for o in range(NCH):
            pp = ps_mm.tile([P, N], F32, tag="mm")
            for c in range(NCH):
                nc.tensor.matmul(pp[:], wp_sb[:, c, o * P:(o + 1) * P],
                                 yT[:, c, :], start=(c == 0), stop=(c == NCH - 1))
            nc.vector.scalar_tensor_tensor(
                x2T[:, o, :], pp[:], bp_sb[:, o:o + 1], xT_s[:, o, :], OP.add, OP.add)
        if dbg:
            nc.gpsimd.dma_start(
                dbg["d_x2T"].rearrange("(c p) t -> p c t", p=P)[:, :, sl], x2T[:])
        return x2T

    def mlp_phase_inner(sb, *a):
        return _mlp_phase_impl(sb, *a)

    def mlp_phase(sb, *a, **kw):
        start = nc.next_id_peek() if hasattr(nc, 'next_id_peek') else None
        PHASE_LOG.append(("mlp_phase", sb, nc.get_next_instruction_name()))
        r = _mlp_phase_impl(sb, *a, **kw)
        return r

    def _mlp_phase_impl(sb, h2T, x2T, only1=False):
        mT = _mlp1_part(sb, h2T)
        if not only1:
            _mlp2_part(sb, mT, x2T)
        return mT

    def _mlp1_part(sb, h2T):
        sl = slice(sb * N, (sb + 1) * N)
        mT = mT_p.tile([P, KCH, N], BF16, tag="m")
        for og in range(KCH // 3):
            w1t = w1_p.tile([P, 3, NCH, P], BF16, tag="w1")
            nc.sync.dma_start(
                w1t[:], w1_a[og * 3:(og + 1) * 3].rearrange("o p c m -> p o c m"))
            for oi in range(3):
                o = og * 3 + oi
                pm = ps_mm.tile([P, N], F32, tag="mm")
                for c in range(NCH):
                    nc.tensor.matmul(pm[:], w1t[:, oi, c, :], h2T[:, c, :],
                                     start=(c == 0), stop=(c == NCH - 1))
                nc.scalar.activation(mT[:, o, :], pm[:], AF.Gelu,
                                     bias=bm1_sb[:, o:o + 1])
        if dbg:
            nc.gpsimd.dma_start(
                dbg["d_mT"].rearrange("(c p) t -> p c t", p=P)[:, :, sl], mT[:])
        return mT

    def _mlp2_part(sb, mT, x2T):
        sl = slice(sb * N, (sb + 1) * N)
        for o in range(NCH):
            w2t = w2_p.tile([P, KCH, P], BF16, tag="w2")
            nc.sync.dma_start(w2t[:], w2_a[o])
            pm2 = ps_acc.tile([P, N], F32, tag="acc")
            for k in range(KCH):
                nc.tensor.matmul(pm2[:], w2t[:, k, :], mT[:, k, :],
                                 start=(k == 0), stop=(k == KCH - 1))
            ot = outp.tile([P, N], F32, tag="o")
            nc.vector.scalar_tensor_tensor(
                ot[:], pm2[:], bm2_sb[:, o:o + 1], x2T[:, o, :], OP.add, OP.add)
            nc.sync.dma_start(out_a[:, o, sl], ot[:])

    # ---- software-pipelined schedule over the S blocks ----
    state = {}
    deferred = {}
    state[0] = ln1_phase(0)
    # big weight loads emitted after the first LN so its input DMA goes first
    nc.sync.dma_start(wv_sb[:], wv_a)
    nc.sync.dma_start(wq_sb[:], wq_a)
    nc.sync.dma_start(wk_sb[:], wk_a)
    nc.sync.dma_start(wp_sb[:], wp_a)
    state[1] = ln1_phase(1)
    yT_cur = attn_phase(0, state[0][1])
    for sb in range(S):
        xT_s, _ = state[sb]
        x2T = proj_phase(sb, yT_cur, xT_s)
        h2T = layernorm(x2T, h2T_p)
        if sb + 2 < S:
            state[sb + 2] = ln1_phase(sb + 2)
        if sb + 1 < S:
            yT_cur = attn_phase(sb + 1, state[sb + 1][1])
        if sb == S - 2:
            deferred[sb] = (mlp_phase(sb, h2T, x2T, only1=True), x2T)
        else:
            if sb == S - 1 and (S - 2) in deferred:
                mTp, x2Tp = deferred.pop(S - 2)
                _mlp2_part(S - 2, mTp, x2Tp)
            mlp_phase(sb, h2T, x2T)

    for p in reversed(list(ctx_pools.values())):
        p.release()


_CACHE = {}


def _get_bass(debug=False, reps=1):
    key = (bool(debug), reps)
    if key not in _CACHE:
        _CACHE[key] = build_bass(debug, reps)
    return _CACHE[key]


def _prep_host(x, g1, b1, Wqkv, Wp, bp, g2, b2, W1, bm1, W2, bm2):
    f32 = np.float32
    bf16 = ml_dtypes.bfloat16
    g1 = np.asarray(g1, f32); b1 = np.asarray(b1, f32)
    Wqkv = np.asarray(Wqkv, f32)
    Wg = Wqkv * g1[:, None]
    bias1 = b1 @ Wqkv
    q, k, v = Wg[:, :DIM], Wg[:, DIM:2 * DIM], Wg[:, 2 * DIM:]
    bias_q, bias_k, bias_v = bias1[:DIM], bias1[DIM:2 * DIM], bias1[2 * DIM:]

    def wlayout(w):  # [DIM, M] -> [P, NCH, M]
        return np.ascontiguousarray(
            w.reshape(NCH, P, -1).transpose(1, 0, 2).astype(bf16))

    def col(b, nch=NCH):  # [nch*P] -> [P, nch]
        return np.ascontiguousarray(b.reshape(nch, P).T.astype(f32))

    g2 = np.asarray(g2, f32); b2 = np.asarray(b2, f32)
    W1 = np.asarray(W1, f32)
    W1g = W1 * g2[:, None]
    bm1_eff = np.asarray(bm1, f32) + b2 @ W1

    # pre-tiled streaming layouts (each [P, chunks, P] tile contiguous)
    w1t = np.ascontiguousarray(
        W1g.reshape(NCH, P, KCH, P).transpose(2, 1, 0, 3).astype(bf16))
    w2t = np.ascontiguousarray(
        np.asarray(W2, f32).reshape(KCH, P, NCH, P).transpose(2, 1, 0, 3).astype(bf16))

    weights = {
        "wq": wlayout(q), "wk": wlayout(k), "wv": wlayout(v),
        "wp": wlayout(np.asarray(Wp, f32)),
        "w1": w1t, "w2": w2t,
        "bq": col(bias_q), "bk": col(bias_k), "bv": col(bias_v),
        "bp": col(np.asarray(bp, f32)),
        "bm1": col(bm1_eff, KCH), "bm2": col(np.asarray(bm2, f32)),
    }
    assert not np.any(bias_v), (
        "nonzero V bias not supported by this kernel build")

    x = np.asarray(x, f32)
    B = x.shape[0]
    xTs = [np.ascontiguousarray(x[c].reshape(TOK, DIM).T) for c in range(B)]
    return weights, xTs


def kernel(x, g1, b1, Wqkv, Wp, bp, g2, b2, W1, bm1, W2, bm2, _debug=False):
    weights, xTs = _prep_host(x, g1, b1, Wqkv, Wp, bp, g2, b2, W1, bm1, W2, bm2)
    nc = _get_bass(_debug)
    in_maps = [dict(weights, xT=xTs[c]) for c in range(8)]
    res = run_bass_kernel_spmd(nc, in_maps, core_ids=list(range(8)))
    outs = []
    for c in range(8):
        o = res.results[c]["out"]          # [DIM, TOK]
        outs.append(np.ascontiguousarray(o.T).reshape(S, N, DIM))
    full = np.stack(outs).astype(np.float32)
    if _debug:
        return full, res
    return full
